# revision 5
# baseline (speedup 1.0000x reference)
"""Trainium2 Bass kernel for LlamaSdpaAttention (B=2, S=2048, D=2048, H=16).

Sharding: 8-way tensor-parallel over heads (2 heads/core, both batches on
every core). Per-core pipeline:
  Phase A(b): QKV projections (fp32r matmuls) + RoPE (perm-matmul + DVE)
  Phase B(b): attention per head: scoresT[sk,sq] matmul, exp on ScalarE,
              softmax denom via ones-matmul, AV matmul -> attn_T[dv,sq],
              normalize with reciprocal
  A2A:        8-rank AllToAll re-shards head-split -> row-split
  Phase C:    output projection for this core's (B*S/8)-row block
Host gathers the 8 row blocks.
"""

import os
import sys
import time

os.environ.setdefault("MYCRO_LOCAL_CACHE", "1")
if "/opt/trn_rl_repo" not in sys.path:
    sys.path.insert(0, "/opt/trn_rl_repo")

import numpy as np

import concourse.bass as bass
import concourse.tile as tile
from concourse import bacc, mybir
from concourse import bass_utils

B = 2
S_FULL = 2048
D = 2048
H = 16
HD = 128
N_CORES = 8
HPC = H // N_CORES          # heads per core = 2
CPC = HPC * HD              # channels per core = 256
ROPE_BASE = 10000.0
F32 = mybir.dt.float32
F32R = mybir.dt.float32r
SCALE = 1.0 / float(np.sqrt(HD))


def _r(ap):
    """View an fp32 AP as float32r for full-rate PE matmuls."""
    return ap.bitcast(F32R)


def build_nc(S=S_FULL):
    """Build the (SPMD, identical-on-all-cores) Bass program."""
    nc = bacc.Bacc("TRN2", target_bir_lowering=False, debug=False,
                   num_devices=N_CORES)

    NSC = S // 512            # s-chunks per batch (4 full, 1 small)
    NKT = S // 128            # sk tiles per batch  (16 full)
    NDT = D // 128            # d (contraction) tiles = 16
    RB = B * S // N_CORES     # rows per core in output (512 full)
    NJ = max(1, 512 // RB)    # a2a row-blocks spanned by one 512-wide sq chunk
    NST = RB // 128           # s-tiles in phase C
    NOR = D // 512            # output column ranges in phase C

    hsT = nc.dram_tensor("hsT", [B, D, S], F32, kind="ExternalInput")
    wqT = nc.dram_tensor("wqT", [D, CPC], F32, kind="ExternalInput")
    wkT = nc.dram_tensor("wkT", [D, CPC], F32, kind="ExternalInput")
    wvT = nc.dram_tensor("wvT", [D, CPC], F32, kind="ExternalInput")
    woT = nc.dram_tensor("woT", [D, D], F32, kind="ExternalInput")
    cosT = nc.dram_tensor("cosT", [HD, S], F32, kind="ExternalInput")
    sinT = nc.dram_tensor("sinT", [HD, S], F32, kind="ExternalInput")
    perm = nc.dram_tensor("perm", [HD, HD], F32, kind="ExternalInput")
    ones = nc.dram_tensor("ones", [128, 1], F32, kind="ExternalInput")
    out = nc.dram_tensor("out", [RB, D], F32, kind="ExternalOutput")

    with tile.TileContext(nc) as tc:
        with (
            tc.tile_pool(name="tiny", bufs=1) as tiny,
            tc.tile_pool(name="mm", bufs=2, space="PSUM") as psum_mm,
            tc.tile_pool(name="ps_s", bufs=2, space="PSUM") as psum_s,
            tc.tile_pool(name="ps_av", bufs=2, space="PSUM") as psum_av,
            tc.tile_pool(name="ps_c", bufs=2, space="PSUM") as psum_c,
            tc.tile_pool(name="dram", bufs=1, space="DRAM") as dram,
        ):
            ones_sb = tiny.tile([128, 1], F32, tag="ones")
            nc.sync.dma_start(out=_r(ones_sb[:]), in_=_r(ones[:, :]))
            a2a_in = dram.tile([N_CORES, CPC, RB], F32, tag="a2a_in")
            a2a_out = dram.tile([N_CORES, CPC, RB], F32, tag="a2a_out")

            hsT_r = [hsT[b, :, :].rearrange("(n p) s -> p n s", p=128)
                     for b in range(B)]

            def phase_a(b, pools, store):
                wq_sb, wk_sb, wv_sb, cos_sb, sin_sb, perm_sb, tmps, hs_pool = pools
                qro, kro, v_sb = store
                for sc in range(NSC):
                    hq = []
                    for qd in range(4):
                        t = hs_pool.tile([128, 4, 512], F32, tag="hs")
                        nc.sync.dma_start(
                            out=_r(t[:]),
                            in_=_r(hsT_r[b][:, qd * 4:(qd + 1) * 4,
                                            sc * 512:(sc + 1) * 512]))
                        hq.append(t)

                    def hstripe(dt):
                        return hq[dt // 4][:, dt % 4, :]

                    for wi, (w_sb, dest) in enumerate(((wq_sb, qro),
                                                      (wk_sb, kro))):
                        for ct in range(HPC):
                            ps = psum_mm.tile([128, 512], F32, tag="mm")
                            for dt in range(NDT):
                                nc.tensor.matmul(
                                    ps[:],
                                    lhsT=_r(w_sb[:, dt, ct * 128:(ct + 1) * 128]),
                                    rhs=_r(hstripe(dt)),
                                    start=(dt == 0), stop=(dt == NDT - 1))
                            qtmp = tmps.tile([128, 512], F32, tag="qtmp")
                            nc.scalar.copy(_r(qtmp[:]), ps[:])
                            # rotate-half via permutation matmul
                            psr = psum_mm.tile([128, 512], F32, tag="mm")
                            nc.tensor.matmul(psr[:], lhsT=_r(perm_sb[:]),
                                             rhs=_r(qtmp[:]),
                                             start=True, stop=True)
                            t1 = tmps.tile([128, 512], F32, tag="t1")
                            nc.vector.tensor_tensor(
                                t1[:], qtmp[:],
                                cos_sb[:, sc * 512:(sc + 1) * 512],
                                mybir.AluOpType.mult)
                            t2 = tmps.tile([128, 512], F32, tag="t2")
                            nc.vector.tensor_tensor(
                                t2[:], psr[:],
                                sin_sb[:, sc * 512:(sc + 1) * 512],
                                mybir.AluOpType.mult)
                            dtile = pools_qkv.tile([128, 512], F32,
                                                   tag=f"qk{wi}_{ct}_{sc}")
                            nc.vector.tensor_tensor(_r(dtile[:]), t1[:],
                                                    t2[:],
                                                    mybir.AluOpType.add)
                            dest[ct][sc] = dtile
                    for st in range(4):
                        ps = psum_mm.tile([128, CPC], F32, tag="mm")
                        for dt in range(NDT):
                            nc.tensor.matmul(
                                ps[:],
                                lhsT=_r(hstripe(dt)[:, st * 128:(st + 1) * 128]),
                                rhs=_r(wv_sb[:, dt, :]),
                                start=(dt == 0), stop=(dt == NDT - 1))
                        vt = pools_qkv.tile([128, CPC], F32,
                                            tag=f"v{sc * 4 + st}")
                        nc.scalar.copy(_r(vt[:]), ps[:])
                        v_sb[sc * 4 + st] = vt

            def phase_b(b, tmps, exps, store):
                qro, kro, v_sb = store
                for h in range(HPC):
                    for sq in range(NSC):
                        ps_c = psum_c.tile([1, 512], F32, tag="ps_c")
                        ps_av = psum_av.tile([128, 512], F32, tag="ps_av")
                        for sk in range(NKT):
                            ps_s = psum_s.tile([128, 512], F32, tag="ps_s")
                            nc.tensor.matmul(
                                ps_s[:],
                                lhsT=_r(kro[h][sk // 4][:, (sk % 4) * 128:(sk % 4 + 1) * 128]),
                                rhs=_r(qro[h][sq][:]),
                                start=True, stop=True)
                            et = exps.tile([128, 512], F32, tag="exp")
                            nc.scalar.activation(
                                _r(et[:]), ps_s[:],
                                mybir.ActivationFunctionType.Exp, scale=SCALE)
                            nc.tensor.matmul(ps_c[:], lhsT=_r(ones_sb[:]),
                                             rhs=_r(et[:]),
                                             start=(sk == 0),
                                             stop=(sk == NKT - 1))
                            nc.tensor.matmul(
                                ps_av[:],
                                lhsT=_r(v_sb[sk][:, h * 128:(h + 1) * 128]),
                                rhs=_r(et[:]),
                                start=(sk == 0), stop=(sk == NKT - 1))
                        rec = tmps.tile([1, 512], F32, tag="rec")
                        nc.vector.reciprocal(rec[:], ps_c[:])
                        bc = tmps.tile([128, 512], F32, tag="bc")
                        nc.gpsimd.partition_broadcast(bc[:], rec[:], 128)
                        attn = tmps.tile([128, 512], F32, tag="attn")
                        nc.vector.tensor_tensor(attn[:], ps_av[:], bc[:],
                                                mybir.AluOpType.mult)
                        j0 = (b * S + sq * 512) // RB
                        for jj in range(NJ):
                            nc.sync.dma_start(
                                out=a2a_in[j0 + jj, h * 128:(h + 1) * 128, :],
                                in_=attn[:, jj * RB:(jj + 1) * RB])

            with (
                tc.tile_pool(name="wsing", bufs=1) as wsing,
                tc.tile_pool(name="tmps", bufs=2) as tmps,
                tc.tile_pool(name="exps", bufs=4) as exps,
            ):
                wq_sb = wsing.tile([128, NDT, CPC], F32, tag="wq")
                wk_sb = wsing.tile([128, NDT, CPC], F32, tag="wk")
                wv_sb = wsing.tile([128, NDT, CPC], F32, tag="wv")
                nc.sync.dma_start(out=_r(wq_sb[:]), in_=_r(wqT[:, :].rearrange("(n p) c -> p n c", p=128)))
                nc.sync.dma_start(out=_r(wk_sb[:]), in_=_r(wkT[:, :].rearrange("(n p) c -> p n c", p=128)))
                nc.sync.dma_start(out=_r(wv_sb[:]), in_=_r(wvT[:, :].rearrange("(n p) c -> p n c", p=128)))
                cos_sb = wsing.tile([HD, S], F32, tag="cos")
                sin_sb = wsing.tile([HD, S], F32, tag="sin")
                nc.sync.dma_start(out=cos_sb[:], in_=cosT[:, :])
                nc.sync.dma_start(out=sin_sb[:], in_=sinT[:, :])
                perm_sb = wsing.tile([HD, HD], F32, tag="perm")
                nc.sync.dma_start(out=_r(perm_sb[:]), in_=_r(perm[:, :]))
                apools = (wq_sb, wk_sb, wv_sb, cos_sb, sin_sb, perm_sb, tmps,
                          None)

                for b in range(B):
                    with (
                        tc.tile_pool(name=f"qkv{b}", bufs=1) as pq,
                        tc.tile_pool(name=f"hs{b}", bufs=4) as hsp,
                    ):
                        global pools_qkv
                        pools_qkv = pq
                        store = ([[None] * NSC for _ in range(HPC)],
                                 [[None] * NSC for _ in range(HPC)],
                                 [None] * NKT)
                        phase_a(b, apools[:7] + (hsp,), store)
                        phase_b(b, tmps, exps, store)

            nc.gpsimd.collective_compute(
                "AllToAll", mybir.AluOpType.bypass,
                replica_groups=[list(range(N_CORES))],
                ins=[a2a_in.opt()], outs=[a2a_out.opt()])

            # ---- Phase C: out[s, o] for this core's RB-row block ----
            with (
                tc.tile_pool(name="wo", bufs=1) as wo_pool,
                tc.tile_pool(name="recv", bufs=1) as recv_pool,
                tc.tile_pool(name="outs", bufs=3) as outs_pool,
            ):
                wo_sb = []
                for ct in range(NDT):
                    t = wo_pool.tile([128, D], F32, tag=f"wo{ct}")
                    nc.sync.dma_start(out=_r(t[:]),
                                      in_=_r(woT[ct * 128:(ct + 1) * 128, :]))
                    wo_sb.append(t)
                rv_sb = []
                for ct in range(NDT):
                    t = recv_pool.tile([128, RB], F32, tag=f"rv{ct}")
                    nc.sync.dma_start(
                        out=_r(t[:]),
                        in_=_r(a2a_out[ct // HPC,
                                       (ct % HPC) * 128:(ct % HPC + 1) * 128, :]))
                    rv_sb.append(t)
                for st in range(NST):
                    for orange in range(NOR):
                        ps = psum_mm.tile([128, 512], F32, tag="mm")
                        for ct in range(NDT):
                            nc.tensor.matmul(
                                ps[:],
                                lhsT=_r(rv_sb[ct][:, st * 128:(st + 1) * 128]),
                                rhs=_r(wo_sb[ct][:, orange * 512:(orange + 1) * 512]),
                                start=(ct == 0), stop=(ct == NDT - 1))
                        osb = outs_pool.tile([128, 512], F32, tag="osb")
                        nc.vector.tensor_copy(out=osb[:], in_=ps[:])
                        nc.sync.dma_start(
                            out=out[st * 128:(st + 1) * 128,
                                    orange * 512:(orange + 1) * 512],
                            in_=osb[:])

    nc.compile()
    return nc


def host_prep(hidden_states, Wq, Wk, Wv, Wo, S=S_FULL):
    """Build per-core input maps (numpy, fp32)."""
    hs = np.asarray(hidden_states, dtype=np.float32)
    hsT = np.ascontiguousarray(hs.transpose(0, 2, 1))          # [B, D, S]
    Wq = np.asarray(Wq, np.float32)
    Wk = np.asarray(Wk, np.float32)
    Wv = np.asarray(Wv, np.float32)
    woT = np.ascontiguousarray(np.asarray(Wo, np.float32).T)   # [D, D]

    inv = 1.0 / (ROPE_BASE ** (np.arange(0, HD, 2, dtype=np.float32) / HD))
    t = np.arange(S, dtype=np.float32)
    freqs = np.outer(t, inv)                                   # [S, HD/2]
    emb = np.concatenate([freqs, freqs], axis=1)               # [S, HD]
    cosT = np.ascontiguousarray(np.cos(emb).T.astype(np.float32))  # [HD, S]
    sinT = np.ascontiguousarray(np.sin(emb).T.astype(np.float32))

    perm = np.zeros((HD, HD), np.float32)   # lhsT of rotate-half: P.T
    idx = np.arange(0, HD, 2)
    perm[idx + 1, idx] = -1.0
    perm[idx, idx + 1] = 1.0

    in_maps = []
    for g in range(N_CORES):
        sl = slice(g * CPC, (g + 1) * CPC)
        in_maps.append({
            "hsT": hsT,
            "wqT": np.ascontiguousarray(Wq[sl, :].T),
            "wkT": np.ascontiguousarray(Wk[sl, :].T),
            "wvT": np.ascontiguousarray(Wv[sl, :].T),
            "woT": woT,
            "cosT": cosT,
            "sinT": sinT,
            "perm": perm,
            "ones": np.ones((128, 1), np.float32),
        })
    return in_maps


_NC_CACHE = {}
LAST_EXEC_SECONDS = None


def kernel(hidden_states, Wq, Wk, Wv, Wo, attention_mask, position_ids):
    global LAST_EXEC_SECONDS
    mask = np.asarray(attention_mask)
    assert mask.all(), "kernel specialized for all-ones attention mask"
    hs = np.asarray(hidden_states, np.float32)
    Bv, S, Dv = hs.shape
    assert (Bv, Dv) == (B, D)

    if S not in _NC_CACHE:
        _NC_CACHE[S] = build_nc(S)
    nc = _NC_CACHE[S]

    in_maps = host_prep(hs, Wq, Wk, Wv, Wo, S=S)
    t0 = time.perf_counter()
    res = bass_utils.run_bass_kernel_spmd(
        nc, in_maps, core_ids=list(range(N_CORES)), trace=False)
    LAST_EXEC_SECONDS = time.perf_counter() - t0

    flat = np.concatenate([res.results[r]["out"] for r in range(N_CORES)],
                          axis=0)                               # [B*S, D]
    return flat.reshape(B, S, Dv).astype(np.float32)
